# revision 28
# baseline (speedup 1.0000x reference)
"""Trainium2 Bass kernel for the biaffine pairwise relation scorer.

Model (per reference):
  h_src = (hidden @ W_src.T + b_src).reshape(B, L, R, H)
  h_tgt = (hidden @ W_tgt.T + b_tgt).reshape(B, L, R, H)
  rel[b, r, s, t]   = sum_h tanh(h_src[b,s,r,h] + h_tgt[b,t,r,h]) * w_out[h]
  mask[b, tt, l]    = sum_h tanh((hidden @ dense_W.T + dense_b)[b,l,tt,h]) * clf_W[0,h] + clf_b

Sharding: 8 cores <-> (b, r) in {0,1} x {0..3}.  Each core computes the full
L x L pairwise block for its (b, r) entirely on-chip (the (B,L,L,R,H)
intermediate never touches HBM).  The tiny dense head is split by h-range
(192 per core within each batch); host sums the partials.

Per-core engine plan:
  PE : projections as N=512 matmuls with hidT stationary (natural [token, rh]
       output), PE transposes for the [h, token] layouts, and the w_out
       contraction as f32r (1 cyc/row) accumulating matmuls.  The four 512-col
       reduce sub-blocks of a source-block land on psum partitions 0/32/64/96
       of ONE bank via zero-padded prefix lhsT tiles (descending-M order), so
       evacuation is a single dense 128-partition copy.
  DVE: pairwise broadcast-add  pair[h, s, t] = srcT[h,s] + tgtT[h,t]
  ACT: tanh over the pair tiles (the throughput floor: ~12.6M elem/core)
"""

import sys

if "/opt/trn_rl_repo" not in sys.path:
    sys.path.insert(0, "/opt/trn_rl_repo")

import numpy as np

B, L, H, R, T = 2, 128, 768, 4, 3
P = 128
KC = H // P            # 6 h-chunks of 128
SB = 16                # source-positions per pairwise block
NSB = L // SB          # 8 blocks
NSUB = 4               # 512-wide reduce sub-blocks per pairwise block
WZ = 97                # prefix-Z lhsT width (w_out at cols 0/32/64/96)
HRANGE = H // 4        # dense-head h columns per core (4 cores per batch)
DROWS = HRANGE * T     # 576 dense rows per core
NZC = (DROWS + P - 1) // P  # 5 z transpose/reduce chunks (4x128 + 64)

_cache = {}


def _build():
    import concourse.bacc as bacc
    import concourse.tile as tile
    from concourse import mybir
    from concourse.masks import make_identity

    f32 = mybir.dt.float32
    f32r = mybir.dt.float32r
    f16 = mybir.dt.float16
    AF = mybir.ActivationFunctionType
    OP = mybir.AluOpType

    nc = bacc.Bacc("TRN2", target_bir_lowering=False, debug=False)

    # ---- DRAM I/O (per-core views; host pre-transposes/pre-tiles) ----
    hidT_d = nc.dram_tensor("hidT", (P, KC * L), f16, kind="ExternalInput")
    wsrcT_d = nc.dram_tensor("wsrcT", (P, KC * H), f16, kind="ExternalInput")
    wtgtT_d = nc.dram_tensor("wtgtT", (P, KC * H), f16, kind="ExternalInput")
    bsrc_d = nc.dram_tensor("bsrc", (1, H), f32, kind="ExternalInput")
    btgt_d = nc.dram_tensor("btgt", (1, H), f32, kind="ExternalInput")
    woutZ_d = nc.dram_tensor("woutZ", (P, KC * WZ), f16, kind="ExternalInput")
    dwT_d = nc.dram_tensor("dwT", (P, KC * DROWS), f16, kind="ExternalInput")
    db_d = nc.dram_tensor("db", (1, DROWS), f32, kind="ExternalInput")
    clfZ_d = nc.dram_tensor("clfZ", (NZC, P, T), f32, kind="ExternalInput")

    rel_d = nc.dram_tensor("rel", (NSB * NSUB, 512), f32, kind="ExternalOutput")
    hsrc_d = nc.dram_tensor("hsrc", (P, H), f32, kind="ExternalOutput")
    htgt_d = nc.dram_tensor("htgt", (P, H), f32, kind="ExternalOutput")
    maskp_d = nc.dram_tensor("maskp", (T, L), f32, kind="ExternalOutput")

    with tile.TileContext(nc) as tc:
        with (
            tc.tile_pool(name="consts", bufs=1) as consts,
            tc.tile_pool(name="wpool", bufs=1) as wpool,
            tc.tile_pool(name="projsb", bufs=1) as projsb,
            tc.tile_pool(name="pairp", bufs=12) as pairp,
            tc.tile_pool(name="evacp", bufs=6) as evacp,
            tc.tile_pool(name="mmps", bufs=2, space="PSUM") as mmps,
            tc.tile_pool(name="rowps", bufs=3, space="PSUM") as rowps,
        ):
            # ---------------- constants + weight DMAs (ring-balanced) ----
            hidT = consts.tile([P, KC * P], f16, name="hidT_sb", tag="hidT_sb")
            nc.sync.dma_start(hidT[:], hidT_d[:])

            wsrc = wpool.tile([P, KC * H], f16, name="wsrc", tag="wsrc")
            wtgt = wpool.tile([P, KC * H], f16, name="wtgt", tag="wtgt")
            engs3 = [nc.sync, nc.scalar, nc.gpsimd]
            qi = 0
            for kc in range(KC):
                for wt, wd in ((wsrc, wsrcT_d), (wtgt, wtgtT_d)):
                    engs3[qi % 3].dma_start(wt[:, kc * H:(kc + 1) * H],
                                            wd[:, kc * H:(kc + 1) * H])
                    qi += 1

            ones = consts.tile([1, P], f32, name="ones", tag="ones")
            nc.vector.memset(ones[:], 1.0)
            ident = consts.tile([P, P], f32, name="ident", tag="ident")
            make_identity(nc, ident[:])
            bsrc = consts.tile([1, H], f32, name="bsrc_sb", tag="bsrc_sb")
            nc.gpsimd.dma_start(bsrc[:], bsrc_d[:])
            btgt = consts.tile([1, H], f32, name="btgt_sb", tag="btgt_sb")
            nc.gpsimd.dma_start(btgt[:], btgt_d[:])
            db = consts.tile([1, DROWS], f32, name="db_sb", tag="db_sb")
            nc.gpsimd.dma_start(db[:], db_d[:])
            woutZ = consts.tile([P, KC * WZ], f16, name="woutZ_sb", tag="woutZ_sb")
            nc.gpsimd.dma_start(woutZ[:], woutZ_d[:])
            clfZ = consts.tile([P, NZC * T], f32, name="clfZ_sb", tag="clfZ_sb")
            for c in range(NZC):
                nc.gpsimd.dma_start(clfZ[:, c * T:(c + 1) * T], clfZ_d[c])

            srcT = projsb.tile([P, KC * P], f32, name="srcT", tag="srcT")
            tgtT = projsb.tile([P, KC * P], f16, name="tgtT", tag="tgtT")
            srcn = projsb.tile([P, H], f32, name="srcn", tag="srcn")
            tgtn = projsb.tile([P, H], f32, name="tgtn", tag="tgtn")
            ztn = projsb.tile([P, DROWS], f32, name="ztn", tag="ztn")
            ztT = projsb.tile([P, NZC * P], f32, name="ztT", tag="ztT")

            dma_engs = [nc.sync, nc.gpsimd, nc.scalar, nc.gpsimd]

            # ---------------- projections (PE, natural layout) ----------
            # h_x[token, rh] = sum_kc hidTr[kc].T @ W_xT[kc]  (+ rank-1 bias)
            src_ps = mmps.tile([P, H], f32, name="src_ps", tag="bigps")
            tgt_ps = mmps.tile([P, H], f32, name="tgt_ps", tag="bigps")
            # column group [0:128] first: unblocks the first pairwise adds
            # (srcT[0]/tgtT[0]) while the remaining columns still project
            for (c0, c1) in ((0, 128), (128, 512), (512, 768)):
                for kc in range(KC):
                    hk = hidT[:, kc * P:(kc + 1) * P]
                    nc.tensor.matmul(src_ps[:, c0:c1], hk,
                                     wsrc[:, kc * H + c0: kc * H + c1],
                                     start=(kc == 0), stop=False)
                    nc.tensor.matmul(tgt_ps[:, c0:c1], hk,
                                     wtgt[:, kc * H + c0: kc * H + c1],
                                     start=(kc == 0), stop=False)
                nc.tensor.matmul(src_ps[:, c0:c1], ones[:], bsrc[:, c0:c1],
                                 start=False, stop=True)
                nc.tensor.matmul(tgt_ps[:, c0:c1], ones[:], btgt[:, c0:c1],
                                 start=False, stop=True)
                nc.vector.tensor_copy(srcn[:, c0:c1], src_ps[:, c0:c1])
                nc.vector.tensor_copy(tgtn[:, c0:c1], tgt_ps[:, c0:c1])
                for kc in range(c0 // P, (c1 + P - 1) // P):
                    for src_of, dst in ((srcn, srcT), (tgtn, tgtT)):
                        tp = rowps.tile([P, P], f32, name=f"tp_{dst.name}{kc}", tag="row")
                        nc.tensor.transpose(tp[:], src_of[:, kc * P:(kc + 1) * P],
                                            ident[:])
                        nc.vector.tensor_copy(dst[:, kc * P:(kc + 1) * P], tp[:])
            nc.sync.dma_start(hsrc_d[:], srcn[:])
            nc.sync.dma_start(htgt_d[:], tgtn[:])

            # ---------------- pairwise (DVE add -> ACT tanh -> PE reduce) ----
            # tapered blocks: small first/last blocks shorten the pipe
            # fill/drain; dense head emitted mid-stream to fill PE gaps
            blocks = []
            s0 = 0
            for sbw in [8, 32, 32, 32, 16, 4, 4]:
                blocks.append((s0, sbw))
                s0 += sbw

            def dense_head():
                z_ps = mmps.tile([P, DROWS], f32, name="z_ps", tag="bigps")
                wd = wpool.tile([P, KC * DROWS], f16, name="wd", tag="wd")
                DB2 = KC * DROWS // 2
                nc.gpsimd.dma_start(wd[:, :DB2], dwT_d[:, :DB2])
                nc.scalar.dma_start(wd[:, DB2:], dwT_d[:, DB2:])
                for kc in range(KC):
                    hk = hidT[:, kc * P:(kc + 1) * P]
                    for n0 in (0, 512):
                        n1 = min(n0 + 512, DROWS)
                        nc.tensor.matmul(z_ps[:, n0:n1], hk,
                                         wd[:, kc * DROWS + n0: kc * DROWS + n1],
                                         start=(kc == 0), stop=False)
                for n0 in (0, 512):
                    n1 = min(n0 + 512, DROWS)
                    nc.tensor.matmul(z_ps[:, n0:n1], ones[:], db[:, n0:n1],
                                     start=False, stop=True)
                nc.scalar.activation(ztn[:], z_ps[:], AF.Tanh)
                for c in range(NZC):
                    w = min(P, DROWS - c * P)
                    tp = rowps.tile([P, P], f32, name=f"tpz{c}", tag="row")
                    nc.tensor.transpose(tp[:w, :], ztn[:, c * P:c * P + w], ident[:])
                    nc.vector.tensor_copy(ztT[:w, c * P:(c + 1) * P], tp[:w, :])
                pm = rowps.tile([T, L], f32, name="pm", tag="row")
                for c in range(NZC):
                    w = min(P, DROWS - c * P)
                    nc.tensor.matmul(pm[:], clfZ[:w, c * T:(c + 1) * T],
                                     ztT[:w, c * P:(c + 1) * P],
                                     start=(c == 0), stop=(c == NZC - 1))
                mev = evacp.tile([T, L], f32, name="mev", tag="ev")
                nc.vector.tensor_copy(mev[:], pm[:])
                nc.sync.dma_start(maskp_d[:], mev[:])

            row0 = 0
            for bi, (s0, sbw) in enumerate(blocks):
                if bi == 2:
                    dense_head()
                nsub = sbw // 4
                ptiles = []
                for kc in range(KC):
                    # 2D tile: the flat contiguous AP lets the fp16 tanh run
                    # in the 2-elem/cycle ScalarE mode
                    pt = pairp.tile([P, sbw * P], f16, name=f"pair{bi}_{kc}", tag="pair")
                    pt3 = pt[:].rearrange("p (a b) -> p a b", b=P)
                    nc.vector.tensor_tensor(
                        pt3,
                        srcT[:, kc * P + s0: kc * P + s0 + sbw][:, :, None]
                        .to_broadcast((P, sbw, P)),
                        tgtT[:, None, kc * P:(kc + 1) * P]
                        .to_broadcast((P, sbw, P)),
                        op=OP.add,
                    )
                    nc.scalar.activation(pt[:], pt[:], AF.Tanh)
                    ptiles.append(pt)
                # Up to four 512-col sub-blocks accumulate in ONE psum bank,
                # on partitions 32j (descending prefix width: later, narrower
                # groups reset the rows the earlier ones smeared).
                for g0 in range(0, sbw * P, 2048):
                    gsub = min(4, (sbw * P - g0) // 512)
                    pp = rowps.tile([P, 512], f32, name=f"pp{bi}_{g0}", tag="row")
                    for j in range(gsub - 1, -1, -1):
                        m = 32 * j + 1
                        for kc in range(KC):
                            nc.tensor.matmul(
                                pp[:m, :], woutZ[:, kc * WZ: kc * WZ + m],
                                ptiles[kc][:, g0 + j * 512: g0 + (j + 1) * 512],
                                start=(kc == 0), stop=(kc == KC - 1),
                            )
                    mtop = 32 * (gsub - 1) + 1
                    ev = evacp.tile([P, 512], f32, name=f"rev{bi}_{g0}", tag="ev")
                    nc.scalar.copy(ev[:mtop, :], pp[:mtop, :])
                    nc.sync.dma_start(rel_d[row0:row0 + gsub, :], ev[0:mtop:32, :])
                    row0 += gsub

    nc.compile()
    return nc


def _in_maps(inputs):
    hidden = np.asarray(inputs["hidden_state"], np.float32)
    W_src = np.asarray(inputs["W_src"], np.float32)
    b_src = np.asarray(inputs["b_src"], np.float32)
    W_tgt = np.asarray(inputs["W_tgt"], np.float32)
    b_tgt = np.asarray(inputs["b_tgt"], np.float32)
    w_out = np.asarray(inputs["w_out"], np.float32)
    dense_W = np.asarray(inputs["dense_W"], np.float32)
    dense_b = np.asarray(inputs["dense_b"], np.float32)
    clf_W = np.asarray(inputs["clf_W"], np.float32)

    def tile_wT(w_block):
        # [rows, H] weight block -> partition-major fp16 [p, kc*rows]
        wT = w_block.T.astype(np.float16)                 # [H(k), rows]
        r = w_block.shape[0]
        return np.ascontiguousarray(
            wT.reshape(KC, P, r).transpose(1, 0, 2).reshape(P, KC * r))

    # prefix-Z lhsT: w_out chunk kc on cols {0,32,64,96} of its 97-block
    woutZ = np.zeros((P, KC * WZ), np.float16)
    for kc in range(KC):
        for j in range(NSUB):
            woutZ[:, kc * WZ + 32 * j] = w_out[kc * P:(kc + 1) * P]

    maps = []
    for c in range(8):
        b, r = c // 4, c % 4
        hr0 = (c % 4) * HRANGE
        rows = np.concatenate(
            [np.arange(tt * H + hr0, tt * H + hr0 + HRANGE) for tt in range(T)]
        )
        clf_slice = clf_W[0, hr0:hr0 + HRANGE]
        clfZ = np.zeros((NZC, P, T), np.float32)
        for zc in range(NZC):
            for p in range(min(P, DROWS - zc * P)):
                row = zc * P + p
                clfZ[zc, p, row // HRANGE] = clf_slice[row % HRANGE]
        maps.append({
            "hidT": np.ascontiguousarray(hidden[b].T.astype(np.float16).reshape(KC, P, L).transpose(1, 0, 2).reshape(P, KC * L)),
            "wsrcT": tile_wT(W_src[r * H:(r + 1) * H]),
            "wtgtT": tile_wT(W_tgt[r * H:(r + 1) * H]),
            "bsrc": np.ascontiguousarray(b_src[r * H:(r + 1) * H]).reshape(1, H),
            "btgt": np.ascontiguousarray(b_tgt[r * H:(r + 1) * H]).reshape(1, H),
            "woutZ": woutZ,
            "dwT": tile_wT(dense_W[rows]),
            "db": np.ascontiguousarray(dense_b[rows]).reshape(1, DROWS),
            "clfZ": clfZ,
        })
    return maps


def _assemble(results, inputs):
    clf_b = np.asarray(inputs["clf_b"], np.float32)
    rel = np.empty((B, R, L, L), np.float32)
    h_src = np.empty((B, L, R, H), np.float32)
    h_tgt = np.empty((B, L, R, H), np.float32)
    mask = np.zeros((B, T, L), np.float32)
    for c in range(8):
        b, r = c // 4, c % 4
        out = results[c]
        rel[b, r] = out["rel"].reshape(L, L)
        h_src[b, :, r, :] = out["hsrc"]
        h_tgt[b, :, r, :] = out["htgt"]
        mask[b] += out["maskp"]
    mask += clf_b[0]
    return rel, mask, h_src, h_tgt


def _run(inputs, trace=False):
    from concourse import bass_utils

    if "nc" not in _cache:
        _cache["nc"] = _build()
    res = bass_utils.run_bass_kernel_spmd(
        _cache["nc"], _in_maps(inputs), core_ids=list(range(8)), trace=trace,
    )
    return _assemble(res.results, inputs), res


def kernel(**inputs):
    out, _ = _run(inputs, trace=False)
    return out


# revision 29
# speedup vs baseline: 1.0500x; 1.0500x over previous
"""Trainium2 Bass kernel for the biaffine pairwise relation scorer.

Model (per reference):
  h_src = (hidden @ W_src.T + b_src).reshape(B, L, R, H)
  h_tgt = (hidden @ W_tgt.T + b_tgt).reshape(B, L, R, H)
  rel[b, r, s, t]   = sum_h tanh(h_src[b,s,r,h] + h_tgt[b,t,r,h]) * w_out[h]
  mask[b, tt, l]    = sum_h tanh((hidden @ dense_W.T + dense_b)[b,l,tt,h]) * clf_W[0,h] + clf_b

Sharding: 8 cores <-> (b, r) in {0,1} x {0..3}.  Each core computes the full
L x L pairwise block for its (b, r) entirely on-chip (the (B,L,L,R,H)
intermediate never touches HBM).  The tiny dense head is split by h-range
(192 per core within each batch); host sums the partials.

Per-core engine plan:
  PE : projections as N=512 matmuls with hidT stationary (natural [token, rh]
       output), PE transposes for the [h, token] layouts, and the w_out
       contraction as f32r (1 cyc/row) accumulating matmuls.  The four 512-col
       reduce sub-blocks of a source-block land on psum partitions 0/32/64/96
       of ONE bank via zero-padded prefix lhsT tiles (descending-M order), so
       evacuation is a single dense 128-partition copy.
  DVE: pairwise broadcast-add  pair[h, s, t] = srcT[h,s] + tgtT[h,t]
  ACT: tanh over the pair tiles (the throughput floor: ~12.6M elem/core)
"""

import sys

if "/opt/trn_rl_repo" not in sys.path:
    sys.path.insert(0, "/opt/trn_rl_repo")

import numpy as np

B, L, H, R, T = 2, 128, 768, 4, 3
P = 128
KC = H // P            # 6 h-chunks of 128
SB = 16                # source-positions per pairwise block
NSB = L // SB          # 8 blocks
NSUB = 4               # 512-wide reduce sub-blocks per pairwise block
WZ = 97                # prefix-Z lhsT width (w_out at cols 0/32/64/96)
HRANGE = H // 4        # dense-head h columns per core (4 cores per batch)
DROWS = HRANGE * T     # 576 dense rows per core
NZC = (DROWS + P - 1) // P  # 5 z transpose/reduce chunks (4x128 + 64)

_cache = {}


def _build():
    import concourse.bacc as bacc
    import concourse.tile as tile
    from concourse import mybir
    from concourse.masks import make_identity

    f32 = mybir.dt.float32
    f32r = mybir.dt.float32r
    f16 = mybir.dt.float16
    AF = mybir.ActivationFunctionType
    OP = mybir.AluOpType

    nc = bacc.Bacc("TRN2", target_bir_lowering=False, debug=False)

    # ---- DRAM I/O (per-core views; host pre-transposes/pre-tiles) ----
    hidT_d = nc.dram_tensor("hidT", (P, KC * L), f16, kind="ExternalInput")
    wsrcT_d = nc.dram_tensor("wsrcT", (P, KC * H), f16, kind="ExternalInput")
    wtgtT_d = nc.dram_tensor("wtgtT", (P, KC * H), f16, kind="ExternalInput")
    bsrc_d = nc.dram_tensor("bsrc", (1, H), f32, kind="ExternalInput")
    btgt_d = nc.dram_tensor("btgt", (1, H), f32, kind="ExternalInput")
    woutZ_d = nc.dram_tensor("woutZ", (P, KC * WZ), f16, kind="ExternalInput")
    dwT_d = nc.dram_tensor("dwT", (P, KC * DROWS), f16, kind="ExternalInput")
    db_d = nc.dram_tensor("db", (1, DROWS), f32, kind="ExternalInput")
    clfZ_d = nc.dram_tensor("clfZ", (NZC, P, T), f32, kind="ExternalInput")

    rel_d = nc.dram_tensor("rel", (NSB * NSUB, 512), f32, kind="ExternalOutput")
    hsrc_d = nc.dram_tensor("hsrc", (P, H), f32, kind="ExternalOutput")
    htgt_d = nc.dram_tensor("htgt", (P, H), f32, kind="ExternalOutput")
    maskp_d = nc.dram_tensor("maskp", (T, L), f32, kind="ExternalOutput")

    with tile.TileContext(nc) as tc:
        with (
            tc.tile_pool(name="consts", bufs=1) as consts,
            tc.tile_pool(name="wpool", bufs=1) as wpool,
            tc.tile_pool(name="projsb", bufs=1) as projsb,
            tc.tile_pool(name="pairp", bufs=12) as pairp,
            tc.tile_pool(name="evacp", bufs=6) as evacp,
            tc.tile_pool(name="mmps", bufs=2, space="PSUM") as mmps,
            tc.tile_pool(name="rowps", bufs=3, space="PSUM") as rowps,
        ):
            # ---------------- constants + weight DMAs (ring-balanced) ----
            hidT = consts.tile([P, KC * P], f16, name="hidT_sb", tag="hidT_sb")
            nc.sync.dma_start(hidT[:], hidT_d[:])

            wsrc = wpool.tile([P, KC * H], f16, name="wsrc", tag="wsrc")
            wtgt = wpool.tile([P, KC * H], f16, name="wtgt", tag="wtgt")
            engs3 = [nc.sync, nc.scalar, nc.gpsimd]
            qi = 0
            for kc in range(KC):
                for wt, wd in ((wsrc, wsrcT_d), (wtgt, wtgtT_d)):
                    engs3[qi % 3].dma_start(wt[:, kc * H:(kc + 1) * H],
                                            wd[:, kc * H:(kc + 1) * H])
                    qi += 1

            ones = consts.tile([1, P], f32, name="ones", tag="ones")
            nc.vector.memset(ones[:], 1.0)
            ident = consts.tile([P, P], f32, name="ident", tag="ident")
            make_identity(nc, ident[:])
            bsrc = consts.tile([1, H], f32, name="bsrc_sb", tag="bsrc_sb")
            nc.gpsimd.dma_start(bsrc[:], bsrc_d[:])
            btgt = consts.tile([1, H], f32, name="btgt_sb", tag="btgt_sb")
            nc.gpsimd.dma_start(btgt[:], btgt_d[:])
            db = consts.tile([1, DROWS], f32, name="db_sb", tag="db_sb")
            nc.gpsimd.dma_start(db[:], db_d[:])
            woutZ = consts.tile([P, KC * WZ], f16, name="woutZ_sb", tag="woutZ_sb")
            nc.gpsimd.dma_start(woutZ[:], woutZ_d[:])
            clfZ = consts.tile([P, NZC * T], f32, name="clfZ_sb", tag="clfZ_sb")
            for c in range(NZC):
                nc.gpsimd.dma_start(clfZ[:, c * T:(c + 1) * T], clfZ_d[c])

            srcT = projsb.tile([P, KC * P], f32, name="srcT", tag="srcT")
            tgtT = projsb.tile([P, KC * P], f16, name="tgtT", tag="tgtT")
            srcn = projsb.tile([P, H], f32, name="srcn", tag="srcn")
            tgtn = projsb.tile([P, H], f32, name="tgtn", tag="tgtn")
            ztn = projsb.tile([P, DROWS], f32, name="ztn", tag="ztn")
            ztT = projsb.tile([P, NZC * P], f32, name="ztT", tag="ztT")

            dma_engs = [nc.sync, nc.gpsimd, nc.scalar, nc.gpsimd]

            # ---------------- projections (PE, natural layout) ----------
            # h_x[token, rh] = sum_kc hidTr[kc].T @ W_xT[kc]  (+ rank-1 bias)
            src_ps = mmps.tile([P, H], f32, name="src_ps", tag="bigps")
            tgt_ps = mmps.tile([P, H], f32, name="tgt_ps", tag="bigps")
            for kc in range(KC):
                hk = hidT[:, kc * P:(kc + 1) * P]
                for n0 in (0, 512):
                    n1 = min(n0 + 512, H)
                    nc.tensor.matmul(src_ps[:, n0:n1], hk,
                                     wsrc[:, kc * H + n0: kc * H + n1],
                                     start=(kc == 0), stop=False)
                    nc.tensor.matmul(tgt_ps[:, n0:n1], hk,
                                     wtgt[:, kc * H + n0: kc * H + n1],
                                     start=(kc == 0), stop=False)
            for n0 in (0, 512):
                n1 = min(n0 + 512, H)
                nc.tensor.matmul(src_ps[:, n0:n1], ones[:], bsrc[:, n0:n1],
                                 start=False, stop=True)
                nc.tensor.matmul(tgt_ps[:, n0:n1], ones[:], btgt[:, n0:n1],
                                 start=False, stop=True)
            nc.vector.tensor_copy(srcn[:], src_ps[:])
            nc.sync.dma_start(hsrc_d[:], srcn[:])
            nc.vector.tensor_copy(tgtn[:], tgt_ps[:])
            nc.sync.dma_start(htgt_d[:], tgtn[:])

            # transpose to [h, token] for the pairwise stage (src/tgt
            # interleaved so the first pairwise block starts early)
            for kc in range(KC):
                for src_of, dst in ((srcn, srcT), (tgtn, tgtT)):
                    tp = rowps.tile([P, P], f32, name=f"tp_{dst.name}{kc}", tag="row")
                    nc.tensor.transpose(tp[:], src_of[:, kc * P:(kc + 1) * P], ident[:])
                    nc.vector.tensor_copy(dst[:, kc * P:(kc + 1) * P], tp[:])

            # ---------------- pairwise (DVE add -> ACT tanh -> PE reduce) ----
            # tapered blocks: small first/last blocks shorten the pipe
            # fill/drain; dense head emitted mid-stream to fill PE gaps
            blocks = []
            s0 = 0
            for sbw in [8, 32, 32, 32, 16, 4, 4]:
                blocks.append((s0, sbw))
                s0 += sbw

            def dense_head():
                z_ps = mmps.tile([P, DROWS], f32, name="z_ps", tag="bigps")
                wd = wpool.tile([P, KC * DROWS], f16, name="wd", tag="wd")
                DB2 = KC * DROWS // 2
                nc.gpsimd.dma_start(wd[:, :DB2], dwT_d[:, :DB2])
                nc.scalar.dma_start(wd[:, DB2:], dwT_d[:, DB2:])
                for kc in range(KC):
                    hk = hidT[:, kc * P:(kc + 1) * P]
                    for n0 in (0, 512):
                        n1 = min(n0 + 512, DROWS)
                        nc.tensor.matmul(z_ps[:, n0:n1], hk,
                                         wd[:, kc * DROWS + n0: kc * DROWS + n1],
                                         start=(kc == 0), stop=False)
                for n0 in (0, 512):
                    n1 = min(n0 + 512, DROWS)
                    nc.tensor.matmul(z_ps[:, n0:n1], ones[:], db[:, n0:n1],
                                     start=False, stop=True)
                nc.scalar.activation(ztn[:], z_ps[:], AF.Tanh)
                for c in range(NZC):
                    w = min(P, DROWS - c * P)
                    tp = rowps.tile([P, P], f32, name=f"tpz{c}", tag="row")
                    nc.tensor.transpose(tp[:w, :], ztn[:, c * P:c * P + w], ident[:])
                    nc.vector.tensor_copy(ztT[:w, c * P:(c + 1) * P], tp[:w, :])
                pm = rowps.tile([T, L], f32, name="pm", tag="row")
                for c in range(NZC):
                    w = min(P, DROWS - c * P)
                    nc.tensor.matmul(pm[:], clfZ[:w, c * T:(c + 1) * T],
                                     ztT[:w, c * P:(c + 1) * P],
                                     start=(c == 0), stop=(c == NZC - 1))
                mev = evacp.tile([T, L], f32, name="mev", tag="ev")
                nc.vector.tensor_copy(mev[:], pm[:])
                nc.sync.dma_start(maskp_d[:], mev[:])

            row0 = 0
            for bi, (s0, sbw) in enumerate(blocks):
                if bi == 2:
                    dense_head()
                nsub = sbw // 4
                ptiles = []
                for kc in range(KC):
                    # 2D tile: the flat contiguous AP lets the fp16 tanh run
                    # in the 2-elem/cycle ScalarE mode
                    pt = pairp.tile([P, sbw * P], f16, name=f"pair{bi}_{kc}", tag="pair")
                    pt3 = pt[:].rearrange("p (a b) -> p a b", b=P)
                    nc.vector.tensor_tensor(
                        pt3,
                        srcT[:, kc * P + s0: kc * P + s0 + sbw][:, :, None]
                        .to_broadcast((P, sbw, P)),
                        tgtT[:, None, kc * P:(kc + 1) * P]
                        .to_broadcast((P, sbw, P)),
                        op=OP.add,
                    )
                    nc.scalar.activation(pt[:], pt[:], AF.Tanh)
                    ptiles.append(pt)
                # Up to four 512-col sub-blocks accumulate in ONE psum bank,
                # on partitions 32j (descending prefix width: later, narrower
                # groups reset the rows the earlier ones smeared).
                for g0 in range(0, sbw * P, 2048):
                    gsub = min(4, (sbw * P - g0) // 512)
                    pp = rowps.tile([P, 512], f32, name=f"pp{bi}_{g0}", tag="row")
                    for j in range(gsub - 1, -1, -1):
                        m = 32 * j + 1
                        for kc in range(KC):
                            nc.tensor.matmul(
                                pp[:m, :], woutZ[:, kc * WZ: kc * WZ + m],
                                ptiles[kc][:, g0 + j * 512: g0 + (j + 1) * 512],
                                start=(kc == 0), stop=(kc == KC - 1),
                            )
                    mtop = 32 * (gsub - 1) + 1
                    ev = evacp.tile([P, 512], f32, name=f"rev{bi}_{g0}", tag="ev")
                    nc.scalar.copy(ev[:mtop, :], pp[:mtop, :])
                    nc.sync.dma_start(rel_d[row0:row0 + gsub, :], ev[0:mtop:32, :])
                    row0 += gsub

    nc.compile()
    return nc


def _in_maps(inputs):
    hidden = np.asarray(inputs["hidden_state"], np.float32)
    W_src = np.asarray(inputs["W_src"], np.float32)
    b_src = np.asarray(inputs["b_src"], np.float32)
    W_tgt = np.asarray(inputs["W_tgt"], np.float32)
    b_tgt = np.asarray(inputs["b_tgt"], np.float32)
    w_out = np.asarray(inputs["w_out"], np.float32)
    dense_W = np.asarray(inputs["dense_W"], np.float32)
    dense_b = np.asarray(inputs["dense_b"], np.float32)
    clf_W = np.asarray(inputs["clf_W"], np.float32)

    def tile_wT(w_block):
        # [rows, H] weight block -> partition-major fp16 [p, kc*rows]
        wT = w_block.T.astype(np.float16)                 # [H(k), rows]
        r = w_block.shape[0]
        return np.ascontiguousarray(
            wT.reshape(KC, P, r).transpose(1, 0, 2).reshape(P, KC * r))

    # prefix-Z lhsT: w_out chunk kc on cols {0,32,64,96} of its 97-block
    woutZ = np.zeros((P, KC * WZ), np.float16)
    for kc in range(KC):
        for j in range(NSUB):
            woutZ[:, kc * WZ + 32 * j] = w_out[kc * P:(kc + 1) * P]

    maps = []
    for c in range(8):
        b, r = c // 4, c % 4
        hr0 = (c % 4) * HRANGE
        rows = np.concatenate(
            [np.arange(tt * H + hr0, tt * H + hr0 + HRANGE) for tt in range(T)]
        )
        clf_slice = clf_W[0, hr0:hr0 + HRANGE]
        clfZ = np.zeros((NZC, P, T), np.float32)
        for zc in range(NZC):
            for p in range(min(P, DROWS - zc * P)):
                row = zc * P + p
                clfZ[zc, p, row // HRANGE] = clf_slice[row % HRANGE]
        maps.append({
            "hidT": np.ascontiguousarray(hidden[b].T.astype(np.float16).reshape(KC, P, L).transpose(1, 0, 2).reshape(P, KC * L)),
            "wsrcT": tile_wT(W_src[r * H:(r + 1) * H]),
            "wtgtT": tile_wT(W_tgt[r * H:(r + 1) * H]),
            "bsrc": np.ascontiguousarray(b_src[r * H:(r + 1) * H]).reshape(1, H),
            "btgt": np.ascontiguousarray(b_tgt[r * H:(r + 1) * H]).reshape(1, H),
            "woutZ": woutZ,
            "dwT": tile_wT(dense_W[rows]),
            "db": np.ascontiguousarray(dense_b[rows]).reshape(1, DROWS),
            "clfZ": clfZ,
        })
    return maps


def _assemble(results, inputs):
    clf_b = np.asarray(inputs["clf_b"], np.float32)
    rel = np.empty((B, R, L, L), np.float32)
    h_src = np.empty((B, L, R, H), np.float32)
    h_tgt = np.empty((B, L, R, H), np.float32)
    mask = np.zeros((B, T, L), np.float32)
    for c in range(8):
        b, r = c // 4, c % 4
        out = results[c]
        rel[b, r] = out["rel"].reshape(L, L)
        h_src[b, :, r, :] = out["hsrc"]
        h_tgt[b, :, r, :] = out["htgt"]
        mask[b] += out["maskp"]
    mask += clf_b[0]
    return rel, mask, h_src, h_tgt


def _run(inputs, trace=False):
    from concourse import bass_utils

    if "nc" not in _cache:
        _cache["nc"] = _build()
    res = bass_utils.run_bass_kernel_spmd(
        _cache["nc"], _in_maps(inputs), core_ids=list(range(8)), trace=trace,
    )
    return _assemble(res.results, inputs), res


def kernel(**inputs):
    out, _ = _run(inputs, trace=False)
    return out
